# revision 35
# baseline (speedup 1.0000x reference)
"""Trainium2 Bass kernel for nn_CiderFeatures (all-pairs Gaussian reduction).

y[i, c] = norms[c] * sum_j exp(-(a_j + b[i,c]) * ||x_i - x_j||^2) * f_j

with per-point scalars a, b, f derived from (rho, gamma, weights).

Strategy (8 NeuronCores, spatially block-sparse row-parallel):
  - The exp argument is a bilinear form: arg[ic, j] = V[:, ic] . U[:, j]
    (10 logical dims expanding -(a_j + b_ic)(r_i + r_j - 2 x_i.x_j)
    + ln f_j + ln norms_c), split into bf16 hi/mid/lo levels (K~54) so
    bf16 matmuls recover ~fp32 precision at full PE speed.
  - Spatial sparsity: the Gaussian factor underflows for all but ~5% of
    pairs.  Points are sorted into 128 tight blocks of 128 via recursive
    coordinate bisection; for each (block, channel) a conservative bound
    keeps only j with exp(-(a_j + bmin_Ic) d2min(I, x_j) + lnf_j + lnn_c)
    >= TAU.  Kept columns are host-gathered into a packed U per core.
  - The 384 (block, channel) units are LPT-balanced across 8 cores x 48
    slots with a shared per-slot width schedule (padded with dead columns
    whose exp underflows to 0), so all cores run one identical program.
  - Device: stream packed U in 2048-col groups (a fused head DMA carries
    the first slots' V plus group 0 to shorten startup); TensorE matmuls
    chopped at PSUM-bank/slot boundaries; ScalarE exp -> fp16 SBUF
    scratch; VectorE in-place halving adds (2x 16-bit mode) + reduce per
    slot-piece; one final 3D reduce -> y.  The trailing groups use the
    ACT accumulator for the j-sum so VectorE does not extend the tail.
"""

import numpy as np
import ml_dtypes
from math import pi, ceil, log

N = 16384
N_CORES = 8
NB = 128                 # spatial blocks
BLK = 128                # points per block (partition dim)
SLOTS = NB * 3 // N_CORES  # 48 (block, channel) units per core
GROUP = 2048             # exp/psum group width (4 PSUM banks)
FIRSTG = 512             # width of the first (startup) group
SECONDG = 1024           # width of the second group
VHEAD = 4 * BLK          # v columns carried by the head DMA (slots 0-3)
BANK = 512               # PSUM bank width in fp32
RTOL2 = 2.6e-4           # keep j for (block, channel) iff some row i has
                         # exp(arg_ij) >= RTOL2 * ytilde_i (exact host test;
                         # ytilde is a per-row lower bound on y)
DEAD_ARG = -300.0        # exponent of dead padding columns (exp -> 0)
LNF_FLOOR = -100.0

SPLIT_LEVELS = 3
MAX_LEVEL_SUM = 2

# keep halving while the pre-halving even length is >= HMIN (each level
# costs ~60ns busy overhead and saves L/2 * 0.52ns of reduce time)
HMIN = 224
# trailing groups whose j-sum uses the ACT accumulator instead of DVE, so
# DVE does not finish last and extend the tail
ACC_TAIL = 2


# ---------------------------------------------------------------------------
# Host math (float64): derived scalars and the bilinear decomposition
# ---------------------------------------------------------------------------

def _derived(rho, gamma, weights, coords):
    A, D = 2.0, 2.0
    B2, C2 = A, (6.0 * pi ** 2) ** (2.0 / 3.0) * (6.0 * A / (160.0 * pi))
    B3, C3 = 2.0 * B2, 2.0 * C2
    B0, C0 = D / A * B2, D / A * C2
    B1, C1 = B2 / 2.0, C2 / 2.0
    Bs = np.array([B0, B1, B2, B3])
    Cs = np.array([C0, C1, C2, C3])
    norms = ((Bs[0] + Bs[1:]) / 2.0) ** 1.5  # (3,)

    rho_ = rho + 1e-8
    t_w = gamma / (8.0 * rho_)
    t_tf = 0.3 * (3.0 * pi ** 2) ** (2.0 / 3.0) * rho_ ** (5.0 / 3.0)
    x = t_w / t_tf
    scale = pi * (rho_ / 2.0) ** (2.0 / 3.0)
    ab = scale[:, None] * (Bs[None, :] + Cs[None, :] * x[:, None])  # (N,4)
    a = ab[:, 0]
    b = ab[:, 1:]                                                   # (N,3)
    f = weights * rho
    lnf = np.log(np.maximum(f, 1e-300))
    lnf = np.maximum(lnf, LNF_FLOOR)
    r = np.sum(coords * coords, axis=1)
    return a, b, f, lnf, r, norms


def _build_vu10(rho, gamma, coords, weights):
    """arg[ic, j] = sum_k V10[k, i, c] * U10[k, j]; a and r mean-centered."""
    a, b, f, lnf, r, norms = _derived(rho, gamma, weights, coords)
    lnn = np.log(norms)
    rbar = float(r.mean())
    rc = r - rbar
    abar = float(a.mean())
    ac = a - abar
    xyz = coords

    V10 = np.empty((10, N, 3))
    U10 = np.empty((10, N))
    V10[0] = np.broadcast_to(rc[:, None], (N, 3))
    U10[0] = -ac
    V10[1] = 1.0
    U10[1] = -a * r + lnf - ac * rbar
    V10[2:5] = np.broadcast_to((2.0 * xyz).T[:, :, None], (3, N, 3))
    U10[2:5] = (ac[:, None] * xyz).T
    V10[5] = b
    U10[5] = -rc
    V10[6] = (-(b * (r[:, None] + rbar))
              - abar * (rc[:, None] + rbar)
              + lnn[None, :])
    U10[6] = 1.0
    V10[7:10] = np.moveaxis(
        2.0 * (b + abar)[:, :, None] * xyz[:, None, :], 2, 0)
    U10[7:10] = xyz.T
    return V10, U10, a, b, lnf, lnn


def _bf16_levels(M, nlev):
    rem = M.copy()
    outs = []
    for _ in range(nlev):
        h = np.asarray(rem, ml_dtypes.bfloat16).astype(np.float64)
        outs.append(h)
        rem = rem - h
    return outs


def build_split_vu(rho, gamma, coords, weights,
                   nlev=SPLIT_LEVELS, max_sum=MAX_LEVEL_SUM):
    """bf16-split V/U plus the scalars needed for the sparsity bound.

    Returns (Vb [K,N,3], Ub [K,N], dead [K], a, b, lnf, lnn) where `dead`
    is a padding U column whose arg is DEAD_ARG for every (i, c).
    """
    V10, U10, a, b, lnf, lnn = _build_vu10(rho, gamma, coords, weights)
    Vlev = [_bf16_levels(V10[d], nlev) for d in range(10)]
    Ulev = [_bf16_levels(U10[d], nlev) for d in range(10)]

    vrows, urows, drows = [], [], []
    for s in range(max_sum + 1):
        for d in range(10):
            for lv in range(min(s, nlev - 1) + 1):
                lu = s - lv
                if lu >= nlev:
                    continue
                v = Vlev[d][lv]
                u = Ulev[d][lu]
                if not v.any() or not u.any():
                    continue
                vrows.append(v)
                urows.append(u)
                # dead col: ride the (d=1, lv=0) pure-j row (V == 1.0)
                drows.append(DEAD_ARG if (d == 1 and lv == 0 and s == 0)
                             else 0.0)
    Vb = np.stack(vrows).astype(np.float32)   # [K, N, 3]
    Ub = np.stack(urows).astype(np.float32)   # [K, N]
    dead = np.array(drows, np.float32)        # [K]
    return Vb, Ub, dead, a, b, lnf, lnn


# ---------------------------------------------------------------------------
# Spatial blocking, sparsity bound, core assignment, schedule
# ---------------------------------------------------------------------------

def _rcb(coords, idx, nblocks):
    if nblocks == 1:
        return [idx]
    pts = coords[idx]
    dim = int(np.argmax(pts.max(0) - pts.min(0)))
    order = np.argsort(pts[:, dim], kind="stable")
    h = len(idx) // 2
    return (_rcb(coords, idx[order[:h]], nblocks // 2)
            + _rcb(coords, idx[order[h:]], nblocks // 2))


def _plan(coords, a, b, lnf, lnn, rtol=RTOL2):
    """Blocks, kept-j lists per (block, channel), core assignment, schedule.

    Two-stage keep test per (block I, channel c):
      1. conservative per-sub-box bbox bound (superset, vectorized)
      2. exact max_i exp(arg_ij) >= rtol * ytilde_i on the candidates

    Returns (blocks, units_per_core, sched) where units_per_core[m] is a
    list of SLOTS tuples (I, c, idx_array) in slot order and sched[s] is
    the shared padded column count of slot s.
    """
    blocks = _rcb(coords, np.arange(N), NB)
    SUB = 16                                  # sub-boxes per block
    SS = BLK // SUB

    # ytilde[i, c]: within-block partial sum — a true lower bound on y
    ytilde = np.zeros((N, 3))
    for blk in blocks:
        d2 = ((coords[blk][:, None, :] - coords[blk][None, :, :]) ** 2).sum(-1)
        for c in range(3):
            w = np.exp(-(a[blk][None, :] + b[blk, c][:, None]) * d2
                       + lnf[blk][None, :] + lnn[c])
            ytilde[blk, c] = w.sum(1)

    r2 = (coords * coords).sum(1)
    units = []
    for I, blk in enumerate(blocks):
        Xi = coords[blk]
        for c in range(3):
            cand = np.zeros(N, bool)
            for s in range(SUB):
                sub = blk[s * SS:(s + 1) * SS]
                T = -log(rtol * ytilde[sub, c].min())
                lo = coords[sub].min(0)
                hi = coords[sub].max(0)
                d = np.maximum(0.0, np.maximum(lo[None, :] - coords,
                                               coords - hi[None, :]))
                d2 = (d * d).sum(1)
                bm = b[sub, c].min()
                cand |= (a + bm) * d2 - lnf - lnn[c] < T
            cand = np.flatnonzero(cand)
            d2 = (r2[blk][:, None] + r2[cand][None, :]
                  - 2.0 * (Xi @ coords[cand].T))
            arg = (-(a[cand][None, :] + b[blk, c][:, None])
                   * np.maximum(d2, 0.0) + lnf[cand][None, :] + lnn[c])
            rel = np.exp(arg) / ytilde[blk, c][:, None]
            units.append((I, c, cand[(rel >= rtol).any(0)]))

    # LPT assignment, capped at SLOTS units per core
    units.sort(key=lambda u: -len(u[2]))
    loads = [0] * N_CORES
    counts = [0] * N_CORES
    percore = [[] for _ in range(N_CORES)]
    for u in units:
        m = min((m for m in range(N_CORES) if counts[m] < SLOTS),
                key=lambda m: loads[m])
        percore[m].append(u)
        loads[m] += len(u[2])
        counts[m] += 1
    # Slot order: valley shape (widest slots at both ends of the stream).
    # Group 0 then only needs the head of v, and the trailing groups have
    # few pieces, keeping the ACT-accumulated tail cheap.
    perm = list(range(0, SLOTS, 2)) + list(range(SLOTS - 1 - (SLOTS % 2), 0, -2))
    for m in range(N_CORES):
        percore[m].sort(key=lambda u: -len(u[2]))
        percore[m] = [percore[m][p] for p in perm]

    sched = []
    for s in range(SLOTS):
        w = max(len(percore[m][s][2]) for m in range(N_CORES))
        sched.append(-16 * (-w // 16))        # round up to multiple of 16
    return blocks, percore, sched


def _pieces(sched):
    """Static program structure from the shared schedule.

    Returns (tot, groups) where groups[g] = (g0, w, mm, red):
      mm  = list of (lo, hi, slot) matmul pieces (group-local, bank-aligned)
      red = list of (lo, hi, pcol) reduce pieces (group-local, slot runs)
    and pcol indexes parts as slot * MAXP + k.
    """
    tot = sum(sched)
    edges = np.cumsum([0] + list(sched))
    # first group is small so the initial DMA + matmul + exp chain starts
    # as early as possible
    bounds = [0, FIRSTG, FIRSTG + SECONDG]
    while bounds[-1] < tot:
        bounds.append(min(bounds[-1] + GROUP, tot))
    bounds = sorted(set(min(b, tot) for b in bounds))
    npieces = [0] * SLOTS
    groups = []
    for g in range(len(bounds) - 1):
        g0, g1 = bounds[g], bounds[g + 1]
        mm, red = [], []
        for s in range(SLOTS):
            lo, hi = max(edges[s], g0), min(edges[s + 1], g1)
            if lo >= hi:
                continue
            red.append((lo - g0, hi - g0, s, npieces[s]))
            npieces[s] += 1
            p = lo
            while p < hi:
                q = min(hi, (p // BANK + 1) * BANK)
                mm.append((p - g0, q - g0, s))
                p = q
        groups.append((g0, g1 - g0, mm, red))
    maxp = max(npieces)
    return tot, groups, maxp


# ---------------------------------------------------------------------------
# Device kernel
# ---------------------------------------------------------------------------

_NC_CACHE = {}


def _build_nc(KK, tot, groups, maxp, repeat=1):
    import concourse.bass as bass  # noqa: F401
    import concourse.tile as tile
    from concourse import bacc, mybir

    nc = bacc.Bacc("TRN2", target_bir_lowering=False)
    # h carries the v columns of slots 0-3 plus group 0's u columns, so a
    # single small DMA unblocks the whole startup chain
    h_dram = nc.dram_tensor("h", [KK, VHEAD + FIRSTG], mybir.dt.bfloat16,
                            kind="ExternalInput")
    u_dram = nc.dram_tensor("u", [KK, tot], mybir.dt.bfloat16,
                            kind="ExternalInput")
    v_dram = nc.dram_tensor("v", [KK, SLOTS * BLK], mybir.dt.bfloat16,
                            kind="ExternalInput")
    y_dram = nc.dram_tensor("y", [BLK, SLOTS], mybir.dt.float32,
                            kind="ExternalOutput")

    def halve(sc, lo, L):
        """In-place halving adds on scratch; returns surviving length."""
        while L % 2 == 0 and L >= HMIN and L >= 2:
            h = L // 2
            nc.vector.add_instruction(
                mybir.InstTensorTensor(
                    name=nc.get_next_instruction_name(),
                    op=mybir.AluOpType.add,
                    ins=[nc.vector.lower_ap(sc[:, lo:lo + h]),
                         nc.vector.lower_ap(sc[:, lo + h:lo + 2 * h])],
                    outs=[nc.vector.lower_ap(sc[:, lo:lo + h])],
                ))
            L = h
        return L

    with tile.TileContext(nc) as tc:
        with (
            tc.tile_pool(name="singles", bufs=1) as singles,
            tc.tile_pool(name="upool", bufs=6) as upool,
            tc.tile_pool(name="psum", bufs=2, space="PSUM") as psum_pool,
            tc.tile_pool(name="scratch", bufs=5) as scratch_pool,
        ):
            # warm the ACT exp table during the input-DMA window
            warm = singles.tile([128, 1], mybir.dt.float32)
            nc.vector.memset(warm[:], 0.0)
            nc.scalar.activation(out=warm[:], in_=warm[:],
                                 func=mybir.ActivationFunctionType.Exp)

            # prime the PE p-state ramp during the same window: ~3us of
            # back-to-back dummy matmuls so the first real groups run at
            # full clock
            prime = singles.tile([KK, 640], mybir.dt.bfloat16)
            nc.vector.memset(prime[:], 0.0)
            pdummy = psum_pool.tile([128, GROUP], mybir.dt.float32, tag="ps")
            for _ in range(4):
                nc.tensor.matmul(pdummy[:, :512], prime[:, :128],
                                 prime[:, 128:640], start=True, stop=True)

            # head DMA (v slots 0-3 + u group 0) goes out first; the v tail
            # streams in two chunks interleaved with the early u groups
            h_sb = singles.tile([KK, VHEAD + FIRSTG], mybir.dt.bfloat16)
            v_sb = singles.tile([KK, SLOTS * BLK], mybir.dt.bfloat16)
            VSPLIT = 16 * BLK
            parts = singles.tile([128, SLOTS, maxp], mybir.dt.float32)
            nc.vector.memset(parts[:], 0.0)
            y_sb = singles.tile([128, SLOTS], mybir.dt.float32)

            def lhs(s):
                if s < 4:
                    return h_sb[:, s * BLK:(s + 1) * BLK]
                return v_sb[:, s * BLK:(s + 1) * BLK]

            for rep in range(repeat):
                u_tiles = [None]
                if rep == 0:
                    nc.sync.dma_start(h_sb[:], h_dram[:])
                for g, (g0, w, mm, red) in enumerate(groups[1:], 1):
                    ut = upool.tile([KK, GROUP], mybir.dt.bfloat16, tag="u")
                    nc.sync.dma_start(ut[:, :w], u_dram[:, g0:g0 + w])
                    u_tiles.append(ut)
                    if rep == 0 and g == 2:
                        nc.sync.dma_start(v_sb[:, VHEAD:VSPLIT],
                                          v_dram[:, VHEAD:VSPLIT])
                    if rep == 0 and g == 3:
                        nc.sync.dma_start(v_sb[:, VSPLIT:],
                                          v_dram[:, VSPLIT:])

                for g, (g0, w, mm, red) in enumerate(groups):
                    pt = psum_pool.tile([128, GROUP], mybir.dt.float32,
                                        tag="ps")
                    for lo, hi, s in mm:
                        rhs = (h_sb[:, VHEAD + lo:VHEAD + hi] if g == 0
                               else u_tiles[g][:, lo:hi])
                        nc.tensor.matmul(pt[:, lo:hi], lhs(s), rhs,
                                         start=True, stop=True)
                    acc_mode = g >= len(groups) - ACC_TAIL
                    if acc_mode:
                        for lo, hi, s, k in red:
                            nc.scalar.activation(
                                out=pt[:, lo:hi], in_=pt[:, lo:hi],
                                func=mybir.ActivationFunctionType.Exp,
                                accum_out=parts[:, s, k:k + 1])
                    else:
                        sc = scratch_pool.tile([128, GROUP],
                                               mybir.dt.float16, tag="sc")
                        nc.scalar.activation(
                            out=sc[:, :w], in_=pt[:, :w],
                            func=mybir.ActivationFunctionType.Exp)
                        for lo, hi, s, k in red:
                            L = halve(sc, lo, hi - lo)
                            nc.vector.reduce_sum(parts[:, s, k:k + 1],
                                                 sc[:, lo:lo + L],
                                                 axis=mybir.AxisListType.X)
            nc.vector.reduce_sum(y_sb[:], parts[:],
                                 axis=mybir.AxisListType.X)
            nc.sync.dma_start(y_dram[:], y_sb[:])
    nc.finalize()
    return nc


# ---------------------------------------------------------------------------
# Host orchestration
# ---------------------------------------------------------------------------

def _prep_inputs(rho, gamma, coords, weights):
    rho = np.asarray(rho, np.float64)
    gamma = np.asarray(gamma, np.float64)
    coords = np.asarray(coords, np.float64)
    weights = np.asarray(weights, np.float64)
    Vb, Ub, dead, a, b, lnf, lnn = build_split_vu(rho, gamma, coords, weights)
    KK = Vb.shape[0]
    blocks, percore, sched = _plan(coords, a, b, lnf, lnn)
    tot, groups, maxp = _pieces(sched)

    Ub16 = Ub.astype(ml_dtypes.bfloat16)
    Vb16 = Vb.astype(ml_dtypes.bfloat16)
    dead16 = dead.astype(ml_dtypes.bfloat16)

    assert sched[0] >= FIRSTG
    in_maps = []
    for m in range(N_CORES):
        u = np.empty((KK, tot), ml_dtypes.bfloat16)
        v = np.empty((KK, SLOTS * BLK), ml_dtypes.bfloat16)
        col = 0
        for s, (I, c, idx) in enumerate(percore[m]):
            w = sched[s]
            u[:, col:col + len(idx)] = Ub16[:, idx]
            u[:, col + len(idx):col + w] = dead16[:, None]
            col += w
            v[:, s * BLK:(s + 1) * BLK] = Vb16[:, blocks[I], c]
        h = np.concatenate([v[:, :VHEAD], u[:, :FIRSTG]], axis=1)
        in_maps.append({"h": np.ascontiguousarray(h), "u": u, "v": v})
    meta = (blocks, percore)
    return KK, tot, groups, maxp, in_maps, meta


def _assemble(results, meta):
    blocks, percore = meta
    out = np.empty((N, 3), np.float32)
    for m, res in enumerate(results):
        y_dev = np.asarray(res["y"])                   # [128, SLOTS]
        for s, (I, c, idx) in enumerate(percore[m]):
            out[blocks[I], c] = y_dev[:, s]
    return out


def kernel_run(rho, gamma, coords, weights, **spmd_kwargs):
    from concourse.bass_utils import run_bass_kernel_spmd

    KK, tot, groups, maxp, in_maps, meta = _prep_inputs(
        rho, gamma, coords, weights)
    key = (KK, tot, maxp, tuple(g[0] for g in groups))
    if key not in _NC_CACHE:
        _NC_CACHE[key] = _build_nc(KK, tot, groups, maxp)
    res = run_bass_kernel_spmd(_NC_CACHE[key], in_maps,
                               core_ids=list(range(N_CORES)), **spmd_kwargs)
    return _assemble(res.results, meta), res


def kernel(rho, gamma, coords, weights):
    y, _ = kernel_run(rho, gamma, coords, weights)
    return y


# revision 47
# speedup vs baseline: 1.0473x; 1.0473x over previous
"""Trainium2 Bass kernel for nn_CiderFeatures (all-pairs Gaussian reduction).

y[i, c] = norms[c] * sum_j exp(-(a_j + b[i,c]) * ||x_i - x_j||^2) * f_j

with per-point scalars a, b, f derived from (rho, gamma, weights).

Strategy (8 NeuronCores, spatially block-sparse row-parallel):
  - The exp argument is a bilinear form: arg[ic, j] = V[:, ic] . U[:, j]
    (10 logical dims expanding -(a_j + b_ic)(r_i + r_j - 2 x_i.x_j)
    + ln f_j + ln norms_c), split into bf16 hi/mid/lo levels (K~54) so
    bf16 matmuls recover ~fp32 precision at full PE speed.
  - Spatial sparsity: the Gaussian factor underflows for all but ~5% of
    pairs.  Points are sorted into 128 tight blocks of 128 via recursive
    coordinate bisection; for each (block, channel) a conservative bound
    keeps only j with exp(-(a_j + bmin_Ic) d2min(I, x_j) + lnf_j + lnn_c)
    >= TAU.  Kept columns are host-gathered into a packed U per core.
  - The 384 (block, channel) units are LPT-balanced across 8 cores x 48
    slots with a shared per-slot width schedule (padded with dead columns
    whose exp underflows to 0), so all cores run one identical program.
  - Device: stream packed U in 2048-col groups (a fused head DMA carries
    the first slots' V plus group 0 to shorten startup); TensorE matmuls
    chopped at PSUM-bank/slot boundaries; ScalarE exp -> fp16 SBUF
    scratch; VectorE in-place halving adds (2x 16-bit mode) + reduce per
    slot-piece; one final 3D reduce -> y.  The trailing groups use the
    ACT accumulator for the j-sum so VectorE does not extend the tail.
"""

import numpy as np
import ml_dtypes
from math import pi, ceil, log

N = 16384
N_CORES = 8
NB = 128                 # spatial blocks
BLK = 128                # points per block (partition dim)
SLOTS = NB * 3 // N_CORES  # 48 (block, channel) units per core
GROUP = 2048             # exp/psum group width (4 PSUM banks)
FIRSTG = 512             # width of the first (startup) group
SECONDG = 1024           # width of the second group
HEADG = 1                # leading groups whose u columns ride the head DMA
HEADU = FIRSTG
VHEAD = 4 * BLK          # v columns carried by the head DMA (slots 0-3)
BANK = 512               # PSUM bank width in fp32
RTOL2 = 2.6e-4           # keep j for (block, channel) iff some row i has
                         # exp(arg_ij) >= RTOL2 * ytilde_i (exact host test;
                         # ytilde is a per-row lower bound on y)
DEAD_ARG = -300.0        # exponent of dead padding columns (exp -> 0)
LNF_FLOOR = -100.0

SPLIT_LEVELS = 3
MAX_LEVEL_SUM = 2

# keep halving while the pre-halving even length is >= HMIN (each level
# costs ~60ns busy overhead and saves L/2 * 0.52ns of reduce time)
HMIN = 224
# trailing groups whose j-sum uses the ACT accumulator instead of DVE, so
# DVE does not finish last and extend the tail
ACC_TAIL = 2
# dummy matmuls that pre-ramp the PE p-state during the startup DMA window
NDUMMY = 4
# slot widths are rounded up to this multiple (keeps halving-add lengths even)
WROUND = 16


# ---------------------------------------------------------------------------
# Host math (float64): derived scalars and the bilinear decomposition
# ---------------------------------------------------------------------------

def _derived(rho, gamma, weights, coords):
    A, D = 2.0, 2.0
    B2, C2 = A, (6.0 * pi ** 2) ** (2.0 / 3.0) * (6.0 * A / (160.0 * pi))
    B3, C3 = 2.0 * B2, 2.0 * C2
    B0, C0 = D / A * B2, D / A * C2
    B1, C1 = B2 / 2.0, C2 / 2.0
    Bs = np.array([B0, B1, B2, B3])
    Cs = np.array([C0, C1, C2, C3])
    norms = ((Bs[0] + Bs[1:]) / 2.0) ** 1.5  # (3,)

    rho_ = rho + 1e-8
    t_w = gamma / (8.0 * rho_)
    t_tf = 0.3 * (3.0 * pi ** 2) ** (2.0 / 3.0) * rho_ ** (5.0 / 3.0)
    x = t_w / t_tf
    scale = pi * (rho_ / 2.0) ** (2.0 / 3.0)
    ab = scale[:, None] * (Bs[None, :] + Cs[None, :] * x[:, None])  # (N,4)
    a = ab[:, 0]
    b = ab[:, 1:]                                                   # (N,3)
    f = weights * rho
    lnf = np.log(np.maximum(f, 1e-300))
    lnf = np.maximum(lnf, LNF_FLOOR)
    r = np.sum(coords * coords, axis=1)
    return a, b, f, lnf, r, norms


def _build_vu10(rho, gamma, coords, weights):
    """arg[ic, j] = sum_k V10[k, i, c] * U10[k, j]; a and r mean-centered."""
    a, b, f, lnf, r, norms = _derived(rho, gamma, weights, coords)
    lnn = np.log(norms)
    rbar = float(r.mean())
    rc = r - rbar
    abar = float(a.mean())
    ac = a - abar
    xyz = coords

    V10 = np.empty((10, N, 3))
    U10 = np.empty((10, N))
    V10[0] = np.broadcast_to(rc[:, None], (N, 3))
    U10[0] = -ac
    V10[1] = 1.0
    U10[1] = -a * r + lnf - ac * rbar
    V10[2:5] = np.broadcast_to((2.0 * xyz).T[:, :, None], (3, N, 3))
    U10[2:5] = (ac[:, None] * xyz).T
    V10[5] = b
    U10[5] = -rc
    V10[6] = (-(b * (r[:, None] + rbar))
              - abar * (rc[:, None] + rbar)
              + lnn[None, :])
    U10[6] = 1.0
    V10[7:10] = np.moveaxis(
        2.0 * (b + abar)[:, :, None] * xyz[:, None, :], 2, 0)
    U10[7:10] = xyz.T
    return V10, U10, a, b, lnf, lnn


def _bf16_levels(M, nlev):
    rem = M.copy()
    outs = []
    for _ in range(nlev):
        h = np.asarray(rem, ml_dtypes.bfloat16).astype(np.float64)
        outs.append(h)
        rem = rem - h
    return outs


def build_split_vu(rho, gamma, coords, weights,
                   nlev=SPLIT_LEVELS, max_sum=MAX_LEVEL_SUM):
    """bf16-split V/U plus the scalars needed for the sparsity bound.

    Returns (Vb [K,N,3], Ub [K,N], dead [K], a, b, lnf, lnn) where `dead`
    is a padding U column whose arg is DEAD_ARG for every (i, c).
    """
    V10, U10, a, b, lnf, lnn = _build_vu10(rho, gamma, coords, weights)
    Vlev = [_bf16_levels(V10[d], nlev) for d in range(10)]
    Ulev = [_bf16_levels(U10[d], nlev) for d in range(10)]

    vrows, urows, drows = [], [], []
    for s in range(max_sum + 1):
        for d in range(10):
            for lv in range(min(s, nlev - 1) + 1):
                lu = s - lv
                if lu >= nlev:
                    continue
                v = Vlev[d][lv]
                u = Ulev[d][lu]
                if not v.any() or not u.any():
                    continue
                vrows.append(v)
                urows.append(u)
                # dead col: ride the (d=1, lv=0) pure-j row (V == 1.0)
                drows.append(DEAD_ARG if (d == 1 and lv == 0 and s == 0)
                             else 0.0)
    Vb = np.stack(vrows).astype(np.float32)   # [K, N, 3]
    Ub = np.stack(urows).astype(np.float32)   # [K, N]
    dead = np.array(drows, np.float32)        # [K]
    return Vb, Ub, dead, a, b, lnf, lnn


# ---------------------------------------------------------------------------
# Spatial blocking, sparsity bound, core assignment, schedule
# ---------------------------------------------------------------------------

def _rcb(coords, idx, nblocks):
    if nblocks == 1:
        return [idx]
    pts = coords[idx]
    dim = int(np.argmax(pts.max(0) - pts.min(0)))
    order = np.argsort(pts[:, dim], kind="stable")
    h = len(idx) // 2
    return (_rcb(coords, idx[order[:h]], nblocks // 2)
            + _rcb(coords, idx[order[h:]], nblocks // 2))


def _plan(coords, a, b, lnf, lnn, rtol=RTOL2):
    """Blocks, kept-j lists per (block, channel), core assignment, schedule.

    Two-stage keep test per (block I, channel c):
      1. conservative per-sub-box bbox bound (superset, vectorized)
      2. exact max_i exp(arg_ij) >= rtol * ytilde_i on the candidates

    Returns (blocks, units_per_core, sched) where units_per_core[m] is a
    list of SLOTS tuples (I, c, idx_array) in slot order and sched[s] is
    the shared padded column count of slot s.
    """
    blocks = _rcb(coords, np.arange(N), NB)
    SUB = 16                                  # sub-boxes per block
    SS = BLK // SUB

    # ytilde[i, c]: within-block partial sum — a true lower bound on y
    ytilde = np.zeros((N, 3))
    for blk in blocks:
        d2 = ((coords[blk][:, None, :] - coords[blk][None, :, :]) ** 2).sum(-1)
        for c in range(3):
            w = np.exp(-(a[blk][None, :] + b[blk, c][:, None]) * d2
                       + lnf[blk][None, :] + lnn[c])
            ytilde[blk, c] = w.sum(1)

    r2 = (coords * coords).sum(1)

    def exact_keep(rows, c, cand):
        d2 = (r2[rows][:, None] + r2[cand][None, :]
              - 2.0 * (coords[rows] @ coords[cand].T))
        arg = (-(a[cand][None, :] + b[rows, c][:, None])
               * np.maximum(d2, 0.0) + lnf[cand][None, :] + lnn[c])
        return (np.exp(arg) / ytilde[rows, c][:, None] >= rtol).any(0)

    def stage1(rows, cb, cn, T):
        """Candidate superset: bound with channel-cb exponents, channel-cn
        norm (valid for any channel c with b_c >= b_cb, lnn_c <= lnn_cn)."""
        keep = np.zeros(N, bool)
        nsub = max(1, len(rows) // SS)
        for s in range(nsub):
            sub = rows[s * SS:(s + 1) * SS]
            d = np.maximum(0.0, np.maximum(coords[sub].min(0)[None, :]
                                           - coords,
                                           coords - coords[sub].max(0)[None, :]))
            d2 = (d * d).sum(1)
            keep |= (a + b[sub, cb].min()) * d2 - lnf - lnn[cn] < T
        return keep

    # Units pack 128 partitions as (point, channel) pairs:
    #  - two units per block of (64-point half) x channels {0, 1} — the
    #    union keep-set spans a smaller radius than full-block units, and
    #    b1 = 2*b0 exactly so the ch0 bound covers ch1
    #  - one unit per block of (128 points) x channel 2
    units = []
    for I, blk in enumerate(blocks):
        for half in (blk[:64], blk[64:]):
            T = -log(rtol * ytilde[half, :2].min())
            cand = np.flatnonzero(stage1(half, 0, 1, T))
            keep = exact_keep(half, 0, cand) | exact_keep(half, 1, cand)
            pidx = np.concatenate([half, half])
            pch = np.array([0] * 64 + [1] * 64)
            units.append((pidx, pch, cand[keep]))
        T = -log(rtol * ytilde[blk, 2].min())
        cand = np.flatnonzero(stage1(blk, 2, 2, T))
        units.append((blk, np.full(BLK, 2), cand[exact_keep(blk, 2, cand)]))

    # LPT assignment, capped at SLOTS units per core
    units.sort(key=lambda u: -len(u[2]))
    loads = [0] * N_CORES
    counts = [0] * N_CORES
    percore = [[] for _ in range(N_CORES)]
    for u in units:
        m = min((m for m in range(N_CORES) if counts[m] < SLOTS),
                key=lambda m: loads[m])
        percore[m].append(u)
        loads[m] += len(u[2])
        counts[m] += 1
    # Slot order: valley shape (widest slots at both ends of the stream).
    # Group 0 then only needs the head of v, and the trailing groups have
    # few pieces, keeping the ACT-accumulated tail cheap.
    perm = list(range(0, SLOTS, 2)) + list(range(SLOTS - 1 - (SLOTS % 2), 0, -2))
    for m in range(N_CORES):
        percore[m].sort(key=lambda u: -len(u[2]))
        percore[m] = [percore[m][p] for p in perm]

    sched = []
    for s in range(SLOTS):
        w = max(len(percore[m][s][2]) for m in range(N_CORES))
        sched.append(-WROUND * (-w // WROUND))
    return blocks, percore, sched


def _pieces(sched):
    """Static program structure from the shared schedule.

    Returns (tot, groups) where groups[g] = (g0, w, mm, red):
      mm  = list of (lo, hi, slot) matmul pieces (group-local, bank-aligned)
      red = list of (lo, hi, pcol) reduce pieces (group-local, slot runs)
    and pcol indexes parts as slot * MAXP + k.
    """
    tot = sum(sched)
    edges = np.cumsum([0] + list(sched))
    # first group is small so the initial DMA + matmul + exp chain starts
    # as early as possible
    bounds = [0, FIRSTG, FIRSTG + SECONDG]
    while bounds[-1] < tot:
        bounds.append(min(bounds[-1] + GROUP, tot))
    bounds = sorted(set(min(b, tot) for b in bounds))
    npieces = [0] * SLOTS
    groups = []
    for g in range(len(bounds) - 1):
        g0, g1 = bounds[g], bounds[g + 1]
        mm, red = [], []
        for s in range(SLOTS):
            lo, hi = max(edges[s], g0), min(edges[s + 1], g1)
            if lo >= hi:
                continue
            red.append((lo - g0, hi - g0, s, npieces[s]))
            npieces[s] += 1
            p = lo
            while p < hi:
                q = min(hi, (p // BANK + 1) * BANK)
                mm.append((p - g0, q - g0, s))
                p = q
        groups.append((g0, g1 - g0, mm, red))
    maxp = max(npieces)
    return tot, groups, maxp


# ---------------------------------------------------------------------------
# Device kernel
# ---------------------------------------------------------------------------

_NC_CACHE = {}


def _build_nc(KK, tot, groups, maxp, repeat=1):
    import concourse.bass as bass  # noqa: F401
    import concourse.tile as tile
    from concourse import bacc, mybir

    nc = bacc.Bacc("TRN2", target_bir_lowering=False)
    # h carries the v columns of slots 0-3 plus the u columns of the first
    # HEADG groups, so a single small DMA unblocks the whole startup chain
    h_dram = nc.dram_tensor("h", [KK, VHEAD + HEADU], mybir.dt.bfloat16,
                            kind="ExternalInput")
    u_dram = nc.dram_tensor("u", [KK, tot], mybir.dt.bfloat16,
                            kind="ExternalInput")
    v_dram = nc.dram_tensor("v", [KK, SLOTS * BLK], mybir.dt.bfloat16,
                            kind="ExternalInput")
    y_dram = nc.dram_tensor("y", [BLK, SLOTS], mybir.dt.float32,
                            kind="ExternalOutput")

    def halve(sc, lo, L):
        """In-place halving adds on scratch; returns surviving length."""
        while L % 2 == 0 and L >= HMIN and L >= 2:
            h = L // 2
            nc.vector.add_instruction(
                mybir.InstTensorTensor(
                    name=nc.get_next_instruction_name(),
                    op=mybir.AluOpType.add,
                    ins=[nc.vector.lower_ap(sc[:, lo:lo + h]),
                         nc.vector.lower_ap(sc[:, lo + h:lo + 2 * h])],
                    outs=[nc.vector.lower_ap(sc[:, lo:lo + h])],
                ))
            L = h
        return L

    with tile.TileContext(nc) as tc:
        with (
            tc.tile_pool(name="singles", bufs=1) as singles,
            tc.tile_pool(name="upool", bufs=6) as upool,
            tc.tile_pool(name="psum", bufs=2, space="PSUM") as psum_pool,
            tc.tile_pool(name="scratch", bufs=5) as scratch_pool,
        ):
            # warm the ACT exp table during the input-DMA window
            warm = singles.tile([128, 1], mybir.dt.float32)
            nc.vector.memset(warm[:], 0.0)
            nc.scalar.activation(out=warm[:], in_=warm[:],
                                 func=mybir.ActivationFunctionType.Exp)

            # prime the PE p-state ramp during the same window: ~3us of
            # back-to-back dummy matmuls so the first real groups run at
            # full clock
            prime = singles.tile([KK, 640], mybir.dt.bfloat16)
            nc.vector.memset(prime[:], 0.0)
            pdummy = psum_pool.tile([128, GROUP], mybir.dt.float32, tag="ps")
            for _ in range(NDUMMY):
                nc.tensor.matmul(pdummy[:, :512], prime[:, :128],
                                 prime[:, 128:640], start=True, stop=True)

            # head DMA (v slots 0-3 + u groups 0..HEADG-1) goes out first;
            # the v tail streams in two chunks interleaved with the u groups
            h_sb = singles.tile([KK, VHEAD + HEADU], mybir.dt.bfloat16)
            v_sb = singles.tile([KK, SLOTS * BLK], mybir.dt.bfloat16)
            VSPLIT = 16 * BLK
            parts = singles.tile([128, SLOTS, maxp], mybir.dt.float32)
            nc.vector.memset(parts[:], 0.0)
            y_sb = singles.tile([128, SLOTS], mybir.dt.float32)

            def lhs(s):
                if s < 4:
                    return h_sb[:, s * BLK:(s + 1) * BLK]
                return v_sb[:, s * BLK:(s + 1) * BLK]

            for rep in range(repeat):
                u_tiles = [None] * HEADG
                if rep == 0:
                    nc.sync.dma_start(h_sb[:], h_dram[:])
                for g, (g0, w, mm, red) in enumerate(groups[HEADG:], HEADG):
                    ut = upool.tile([KK, GROUP], mybir.dt.bfloat16, tag="u")
                    nc.sync.dma_start(ut[:, :w], u_dram[:, g0:g0 + w])
                    u_tiles.append(ut)
                    if rep == 0 and g == HEADG + 1:
                        nc.sync.dma_start(v_sb[:, VHEAD:VSPLIT],
                                          v_dram[:, VHEAD:VSPLIT])
                    if rep == 0 and g == HEADG + 2:
                        nc.sync.dma_start(v_sb[:, VSPLIT:],
                                          v_dram[:, VSPLIT:])

                for g, (g0, w, mm, red) in enumerate(groups):
                    pt = psum_pool.tile([128, GROUP], mybir.dt.float32,
                                        tag="ps")
                    for lo, hi, s in mm:
                        rhs = (h_sb[:, VHEAD + g0 + lo:VHEAD + g0 + hi]
                               if g < HEADG else u_tiles[g][:, lo:hi])
                        nc.tensor.matmul(pt[:, lo:hi], lhs(s), rhs,
                                         start=True, stop=True)
                    acc_mode = g >= len(groups) - ACC_TAIL
                    if acc_mode:
                        for lo, hi, s, k in red:
                            nc.scalar.activation(
                                out=pt[:, lo:hi], in_=pt[:, lo:hi],
                                func=mybir.ActivationFunctionType.Exp,
                                accum_out=parts[:, s, k:k + 1])
                    else:
                        sc = scratch_pool.tile([128, GROUP],
                                               mybir.dt.float16, tag="sc")
                        nc.scalar.activation(
                            out=sc[:, :w], in_=pt[:, :w],
                            func=mybir.ActivationFunctionType.Exp)
                        for lo, hi, s, k in red:
                            L = halve(sc, lo, hi - lo)
                            nc.vector.reduce_sum(parts[:, s, k:k + 1],
                                                 sc[:, lo:lo + L],
                                                 axis=mybir.AxisListType.X)
            nc.vector.reduce_sum(y_sb[:], parts[:],
                                 axis=mybir.AxisListType.X)
            nc.sync.dma_start(y_dram[:], y_sb[:])
    nc.finalize()
    return nc


# ---------------------------------------------------------------------------
# Host orchestration
# ---------------------------------------------------------------------------

def _prep_inputs(rho, gamma, coords, weights):
    rho = np.asarray(rho, np.float64)
    gamma = np.asarray(gamma, np.float64)
    coords = np.asarray(coords, np.float64)
    weights = np.asarray(weights, np.float64)
    Vb, Ub, dead, a, b, lnf, lnn = build_split_vu(rho, gamma, coords, weights)
    KK = Vb.shape[0]
    blocks, percore, sched = _plan(coords, a, b, lnf, lnn)
    tot, groups, maxp = _pieces(sched)

    Ub16 = Ub.astype(ml_dtypes.bfloat16)
    Vb16 = Vb.astype(ml_dtypes.bfloat16)
    dead16 = dead.astype(ml_dtypes.bfloat16)

    assert sched[0] >= FIRSTG
    in_maps = []
    for m in range(N_CORES):
        u = np.empty((KK, tot), ml_dtypes.bfloat16)
        v = np.empty((KK, SLOTS * BLK), ml_dtypes.bfloat16)
        col = 0
        for s, (pidx, pch, idx) in enumerate(percore[m]):
            w = sched[s]
            u[:, col:col + len(idx)] = Ub16[:, idx]
            u[:, col + len(idx):col + w] = dead16[:, None]
            col += w
            v[:, s * BLK:(s + 1) * BLK] = Vb16[:, pidx, pch]
        h = np.concatenate([v[:, :VHEAD], u[:, :HEADU]], axis=1)
        in_maps.append({"h": np.ascontiguousarray(h), "u": u, "v": v})
    meta = (blocks, percore)
    return KK, tot, groups, maxp, in_maps, meta


def _assemble(results, meta):
    blocks, percore = meta
    out = np.empty((N, 3), np.float32)
    for m, res in enumerate(results):
        y_dev = np.asarray(res["y"])                   # [128, SLOTS]
        for s, (pidx, pch, idx) in enumerate(percore[m]):
            out[pidx, pch] = y_dev[:, s]
    return out


def kernel_run(rho, gamma, coords, weights, **spmd_kwargs):
    from concourse.bass_utils import run_bass_kernel_spmd

    KK, tot, groups, maxp, in_maps, meta = _prep_inputs(
        rho, gamma, coords, weights)
    key = (KK, tot, maxp, tuple(g[0] for g in groups))
    if key not in _NC_CACHE:
        _NC_CACHE[key] = _build_nc(KK, tot, groups, maxp)
    res = run_bass_kernel_spmd(_NC_CACHE[key], in_maps,
                               core_ids=list(range(N_CORES)), **spmd_kwargs)
    return _assemble(res.results, meta), res


def kernel(rho, gamma, coords, weights):
    y, _ = kernel_run(rho, gamma, coords, weights)
    return y


# revision 49
# speedup vs baseline: 1.0665x; 1.0184x over previous
"""Trainium2 Bass kernel for nn_CiderFeatures (all-pairs Gaussian reduction).

y[i, c] = norms[c] * sum_j exp(-(a_j + b[i,c]) * ||x_i - x_j||^2) * f_j

with per-point scalars a, b, f derived from (rho, gamma, weights).

Strategy (8 NeuronCores, spatially block-sparse row-parallel):
  - The exp argument is a bilinear form: arg[ic, j] = V[:, ic] . U[:, j]
    (10 logical dims expanding -(a_j + b_ic)(r_i + r_j - 2 x_i.x_j)
    + ln f_j + ln norms_c), split into bf16 hi/mid/lo levels (K~54) so
    bf16 matmuls recover ~fp32 precision at full PE speed.
  - Spatial sparsity: the Gaussian factor underflows for all but ~5% of
    pairs.  Points are sorted into 128 tight blocks of 128 via recursive
    coordinate bisection; for each (block, channel) a conservative bound
    keeps only j with exp(-(a_j + bmin_Ic) d2min(I, x_j) + lnf_j + lnn_c)
    >= TAU.  Kept columns are host-gathered into a packed U per core.
  - The 384 (block, channel) units are LPT-balanced across 8 cores x 48
    slots with a shared per-slot width schedule (padded with dead columns
    whose exp underflows to 0), so all cores run one identical program.
  - Device: stream packed U in 2048-col groups (a fused head DMA carries
    the first slots' V plus group 0 to shorten startup); TensorE matmuls
    chopped at PSUM-bank/slot boundaries; ScalarE exp -> fp16 SBUF
    scratch; VectorE in-place halving adds (2x 16-bit mode) + reduce per
    slot-piece; one final 3D reduce -> y.  The trailing groups use the
    ACT accumulator for the j-sum so VectorE does not extend the tail.
"""

import numpy as np
import ml_dtypes
from math import pi, ceil, log

N = 16384
N_CORES = 8
NB = 128                 # spatial blocks
BLK = 128                # points per block (partition dim)
SLOTS = NB * 3 // N_CORES  # 48 (block, channel) units per core
GROUP = 2048             # exp/psum group width (4 PSUM banks)
FIRSTG = 512             # width of the first (startup) group
SECONDG = 1024           # width of the second group
HEADG = 1                # leading groups whose u columns ride the head DMA
HEADU = FIRSTG
VHEAD = 4 * BLK          # v columns carried by the head DMA (slots 0-3)
BANK = 512               # PSUM bank width in fp32
RTOL2 = 2.8e-4           # keep j for (block, channel) iff some row i has
                         # exp(arg_ij) >= RTOL2 * ytilde_i (exact host test;
                         # ytilde is a per-row lower bound on y)
DEAD_ARG = -300.0        # exponent of dead padding columns (exp -> 0)
LNF_FLOOR = -100.0

SPLIT_LEVELS = 3
MAX_LEVEL_SUM = 2

# keep halving while the pre-halving even length is >= HMIN (each level
# costs ~60ns busy overhead and saves L/2 * 0.52ns of reduce time)
HMIN = 224
# trailing groups whose j-sum uses the ACT accumulator instead of DVE, so
# DVE does not finish last and extend the tail
ACC_TAIL = 3
# dummy matmuls that pre-ramp the PE p-state during the startup DMA window
NDUMMY = 4
# slot widths are rounded up to this multiple (keeps halving-add lengths even)
WROUND = 16


# ---------------------------------------------------------------------------
# Host math (float64): derived scalars and the bilinear decomposition
# ---------------------------------------------------------------------------

def _derived(rho, gamma, weights, coords):
    A, D = 2.0, 2.0
    B2, C2 = A, (6.0 * pi ** 2) ** (2.0 / 3.0) * (6.0 * A / (160.0 * pi))
    B3, C3 = 2.0 * B2, 2.0 * C2
    B0, C0 = D / A * B2, D / A * C2
    B1, C1 = B2 / 2.0, C2 / 2.0
    Bs = np.array([B0, B1, B2, B3])
    Cs = np.array([C0, C1, C2, C3])
    norms = ((Bs[0] + Bs[1:]) / 2.0) ** 1.5  # (3,)

    rho_ = rho + 1e-8
    t_w = gamma / (8.0 * rho_)
    t_tf = 0.3 * (3.0 * pi ** 2) ** (2.0 / 3.0) * rho_ ** (5.0 / 3.0)
    x = t_w / t_tf
    scale = pi * (rho_ / 2.0) ** (2.0 / 3.0)
    ab = scale[:, None] * (Bs[None, :] + Cs[None, :] * x[:, None])  # (N,4)
    a = ab[:, 0]
    b = ab[:, 1:]                                                   # (N,3)
    f = weights * rho
    lnf = np.log(np.maximum(f, 1e-300))
    lnf = np.maximum(lnf, LNF_FLOOR)
    r = np.sum(coords * coords, axis=1)
    return a, b, f, lnf, r, norms


def _build_vu10(rho, gamma, coords, weights):
    """arg[ic, j] = sum_k V10[k, i, c] * U10[k, j]; a and r mean-centered."""
    a, b, f, lnf, r, norms = _derived(rho, gamma, weights, coords)
    lnn = np.log(norms)
    rbar = float(r.mean())
    rc = r - rbar
    abar = float(a.mean())
    ac = a - abar
    xyz = coords

    V10 = np.empty((10, N, 3))
    U10 = np.empty((10, N))
    V10[0] = np.broadcast_to(rc[:, None], (N, 3))
    U10[0] = -ac
    V10[1] = 1.0
    U10[1] = -a * r + lnf - ac * rbar
    V10[2:5] = np.broadcast_to((2.0 * xyz).T[:, :, None], (3, N, 3))
    U10[2:5] = (ac[:, None] * xyz).T
    V10[5] = b
    U10[5] = -rc
    V10[6] = (-(b * (r[:, None] + rbar))
              - abar * (rc[:, None] + rbar)
              + lnn[None, :])
    U10[6] = 1.0
    V10[7:10] = np.moveaxis(
        2.0 * (b + abar)[:, :, None] * xyz[:, None, :], 2, 0)
    U10[7:10] = xyz.T
    return V10, U10, a, b, lnf, lnn


def _bf16_levels(M, nlev):
    rem = M.copy()
    outs = []
    for _ in range(nlev):
        h = np.asarray(rem, ml_dtypes.bfloat16).astype(np.float64)
        outs.append(h)
        rem = rem - h
    return outs


def build_split_vu(rho, gamma, coords, weights,
                   nlev=SPLIT_LEVELS, max_sum=MAX_LEVEL_SUM):
    """bf16-split V/U plus the scalars needed for the sparsity bound.

    Returns (Vb [K,N,3], Ub [K,N], dead [K], a, b, lnf, lnn) where `dead`
    is a padding U column whose arg is DEAD_ARG for every (i, c).
    """
    V10, U10, a, b, lnf, lnn = _build_vu10(rho, gamma, coords, weights)
    Vlev = [_bf16_levels(V10[d], nlev) for d in range(10)]
    Ulev = [_bf16_levels(U10[d], nlev) for d in range(10)]

    vrows, urows, drows = [], [], []
    for s in range(max_sum + 1):
        for d in range(10):
            for lv in range(min(s, nlev - 1) + 1):
                lu = s - lv
                if lu >= nlev:
                    continue
                v = Vlev[d][lv]
                u = Ulev[d][lu]
                if not v.any() or not u.any():
                    continue
                vrows.append(v)
                urows.append(u)
                # dead col: ride the (d=1, lv=0) pure-j row (V == 1.0)
                drows.append(DEAD_ARG if (d == 1 and lv == 0 and s == 0)
                             else 0.0)
    Vb = np.stack(vrows).astype(np.float32)   # [K, N, 3]
    Ub = np.stack(urows).astype(np.float32)   # [K, N]
    dead = np.array(drows, np.float32)        # [K]
    return Vb, Ub, dead, a, b, lnf, lnn


# ---------------------------------------------------------------------------
# Spatial blocking, sparsity bound, core assignment, schedule
# ---------------------------------------------------------------------------

def _rcb(coords, idx, nblocks):
    if nblocks == 1:
        return [idx]
    pts = coords[idx]
    dim = int(np.argmax(pts.max(0) - pts.min(0)))
    order = np.argsort(pts[:, dim], kind="stable")
    h = len(idx) // 2
    return (_rcb(coords, idx[order[:h]], nblocks // 2)
            + _rcb(coords, idx[order[h:]], nblocks // 2))


def _plan(coords, a, b, lnf, lnn, rtol=RTOL2):
    """Blocks, kept-j lists per (block, channel), core assignment, schedule.

    Two-stage keep test per (block I, channel c):
      1. conservative per-sub-box bbox bound (superset, vectorized)
      2. exact max_i exp(arg_ij) >= rtol * ytilde_i on the candidates

    Returns (blocks, units_per_core, sched) where units_per_core[m] is a
    list of SLOTS tuples (I, c, idx_array) in slot order and sched[s] is
    the shared padded column count of slot s.
    """
    blocks = _rcb(coords, np.arange(N), NB)
    SUB = 16                                  # sub-boxes per block
    SS = BLK // SUB

    # ytilde[i, c]: within-block partial sum — a true lower bound on y
    ytilde = np.zeros((N, 3))
    for blk in blocks:
        d2 = ((coords[blk][:, None, :] - coords[blk][None, :, :]) ** 2).sum(-1)
        for c in range(3):
            w = np.exp(-(a[blk][None, :] + b[blk, c][:, None]) * d2
                       + lnf[blk][None, :] + lnn[c])
            ytilde[blk, c] = w.sum(1)

    r2 = (coords * coords).sum(1)

    def exact_keep(rows, c, cand):
        d2 = (r2[rows][:, None] + r2[cand][None, :]
              - 2.0 * (coords[rows] @ coords[cand].T))
        arg = (-(a[cand][None, :] + b[rows, c][:, None])
               * np.maximum(d2, 0.0) + lnf[cand][None, :] + lnn[c])
        return (np.exp(arg) / ytilde[rows, c][:, None] >= rtol).any(0)

    def stage1(rows, cb, cn, T):
        """Candidate superset: bound with channel-cb exponents, channel-cn
        norm (valid for any channel c with b_c >= b_cb, lnn_c <= lnn_cn)."""
        keep = np.zeros(N, bool)
        nsub = max(1, len(rows) // SS)
        for s in range(nsub):
            sub = rows[s * SS:(s + 1) * SS]
            d = np.maximum(0.0, np.maximum(coords[sub].min(0)[None, :]
                                           - coords,
                                           coords - coords[sub].max(0)[None, :]))
            d2 = (d * d).sum(1)
            keep |= (a + b[sub, cb].min()) * d2 - lnf - lnn[cn] < T
        return keep

    # Units pack 128 partitions as (point, channel) pairs:
    #  - two units per block of (64-point half) x channels {0, 1} — the
    #    union keep-set spans a smaller radius than full-block units, and
    #    b1 = 2*b0 exactly so the ch0 bound covers ch1
    #  - one unit per block of (128 points) x channel 2
    units = []
    for I, blk in enumerate(blocks):
        for half in (blk[:64], blk[64:]):
            T = -log(rtol * ytilde[half, :2].min())
            cand = np.flatnonzero(stage1(half, 0, 1, T))
            keep = exact_keep(half, 0, cand) | exact_keep(half, 1, cand)
            pidx = np.concatenate([half, half])
            pch = np.array([0] * 64 + [1] * 64)
            units.append((pidx, pch, cand[keep]))
        T = -log(rtol * ytilde[blk, 2].min())
        cand = np.flatnonzero(stage1(blk, 2, 2, T))
        units.append((blk, np.full(BLK, 2), cand[exact_keep(blk, 2, cand)]))

    # LPT assignment, capped at SLOTS units per core
    units.sort(key=lambda u: -len(u[2]))
    loads = [0] * N_CORES
    counts = [0] * N_CORES
    percore = [[] for _ in range(N_CORES)]
    for u in units:
        m = min((m for m in range(N_CORES) if counts[m] < SLOTS),
                key=lambda m: loads[m])
        percore[m].append(u)
        loads[m] += len(u[2])
        counts[m] += 1
    # Slot order: valley shape (widest slots at both ends of the stream).
    # Group 0 then only needs the head of v, and the trailing groups have
    # few pieces, keeping the ACT-accumulated tail cheap.
    perm = list(range(0, SLOTS, 2)) + list(range(SLOTS - 1 - (SLOTS % 2), 0, -2))
    for m in range(N_CORES):
        percore[m].sort(key=lambda u: -len(u[2]))
        percore[m] = [percore[m][p] for p in perm]

    sched = []
    for s in range(SLOTS):
        w = max(len(percore[m][s][2]) for m in range(N_CORES))
        sched.append(-WROUND * (-w // WROUND))
    return blocks, percore, sched


def _pieces(sched):
    """Static program structure from the shared schedule.

    Returns (tot, groups) where groups[g] = (g0, w, mm, red):
      mm  = list of (lo, hi, slot) matmul pieces (group-local, bank-aligned)
      red = list of (lo, hi, pcol) reduce pieces (group-local, slot runs)
    and pcol indexes parts as slot * MAXP + k.
    """
    tot = sum(sched)
    edges = np.cumsum([0] + list(sched))
    # first group is small so the initial DMA + matmul + exp chain starts
    # as early as possible
    bounds = [0, FIRSTG, FIRSTG + SECONDG]
    while bounds[-1] < tot:
        bounds.append(min(bounds[-1] + GROUP, tot))
    bounds = sorted(set(min(b, tot) for b in bounds))
    npieces = [0] * SLOTS
    groups = []
    for g in range(len(bounds) - 1):
        g0, g1 = bounds[g], bounds[g + 1]
        mm, red = [], []
        for s in range(SLOTS):
            lo, hi = max(edges[s], g0), min(edges[s + 1], g1)
            if lo >= hi:
                continue
            red.append((lo - g0, hi - g0, s, npieces[s]))
            npieces[s] += 1
            p = lo
            while p < hi:
                q = min(hi, (p // BANK + 1) * BANK)
                mm.append((p - g0, q - g0, s))
                p = q
        groups.append((g0, g1 - g0, mm, red))
    maxp = max(npieces)
    return tot, groups, maxp


# ---------------------------------------------------------------------------
# Device kernel
# ---------------------------------------------------------------------------

_NC_CACHE = {}


def _build_nc(KK, tot, groups, maxp, repeat=1):
    import concourse.bass as bass  # noqa: F401
    import concourse.tile as tile
    from concourse import bacc, mybir

    nc = bacc.Bacc("TRN2", target_bir_lowering=False)
    # h carries the v columns of slots 0-3 plus the u columns of the first
    # HEADG groups, so a single small DMA unblocks the whole startup chain
    h_dram = nc.dram_tensor("h", [KK, VHEAD + HEADU], mybir.dt.bfloat16,
                            kind="ExternalInput")
    u_dram = nc.dram_tensor("u", [KK, tot], mybir.dt.bfloat16,
                            kind="ExternalInput")
    v_dram = nc.dram_tensor("v", [KK, SLOTS * BLK], mybir.dt.bfloat16,
                            kind="ExternalInput")
    y_dram = nc.dram_tensor("y", [BLK, SLOTS], mybir.dt.float32,
                            kind="ExternalOutput")

    def halve(sc, lo, L):
        """In-place halving adds on scratch; returns surviving length."""
        while L % 2 == 0 and L >= HMIN and L >= 2:
            h = L // 2
            nc.vector.add_instruction(
                mybir.InstTensorTensor(
                    name=nc.get_next_instruction_name(),
                    op=mybir.AluOpType.add,
                    ins=[nc.vector.lower_ap(sc[:, lo:lo + h]),
                         nc.vector.lower_ap(sc[:, lo + h:lo + 2 * h])],
                    outs=[nc.vector.lower_ap(sc[:, lo:lo + h])],
                ))
            L = h
        return L

    with tile.TileContext(nc) as tc:
        with (
            tc.tile_pool(name="singles", bufs=1) as singles,
            tc.tile_pool(name="upool", bufs=6) as upool,
            tc.tile_pool(name="psum", bufs=2, space="PSUM") as psum_pool,
            tc.tile_pool(name="scratch", bufs=5) as scratch_pool,
        ):
            # warm the ACT exp table during the input-DMA window
            warm = singles.tile([128, 1], mybir.dt.float32)
            nc.vector.memset(warm[:], 0.0)
            nc.scalar.activation(out=warm[:], in_=warm[:],
                                 func=mybir.ActivationFunctionType.Exp)

            # prime the PE p-state ramp during the same window: ~3us of
            # back-to-back dummy matmuls so the first real groups run at
            # full clock
            prime = singles.tile([KK, 640], mybir.dt.bfloat16)
            nc.vector.memset(prime[:], 0.0)
            pdummy = psum_pool.tile([128, GROUP], mybir.dt.float32, tag="ps")
            for _ in range(NDUMMY):
                nc.tensor.matmul(pdummy[:, :512], prime[:, :128],
                                 prime[:, 128:640], start=True, stop=True)

            # head DMA (v slots 0-3 + u groups 0..HEADG-1) goes out first;
            # the v tail streams in two chunks interleaved with the u groups
            h_sb = singles.tile([KK, VHEAD + HEADU], mybir.dt.bfloat16)
            v_sb = singles.tile([KK, SLOTS * BLK], mybir.dt.bfloat16)
            VSPLIT = 16 * BLK
            parts = singles.tile([128, SLOTS, maxp], mybir.dt.float32)
            nc.vector.memset(parts[:], 0.0)
            y_sb = singles.tile([128, SLOTS], mybir.dt.float32)

            def lhs(s):
                if s < 4:
                    return h_sb[:, s * BLK:(s + 1) * BLK]
                return v_sb[:, s * BLK:(s + 1) * BLK]

            for rep in range(repeat):
                u_tiles = [None] * HEADG
                if rep == 0:
                    nc.sync.dma_start(h_sb[:], h_dram[:])
                for g, (g0, w, mm, red) in enumerate(groups[HEADG:], HEADG):
                    ut = upool.tile([KK, GROUP], mybir.dt.bfloat16, tag="u")
                    nc.sync.dma_start(ut[:, :w], u_dram[:, g0:g0 + w])
                    u_tiles.append(ut)
                    if rep == 0 and g == HEADG + 1:
                        nc.sync.dma_start(v_sb[:, VHEAD:VSPLIT],
                                          v_dram[:, VHEAD:VSPLIT])
                    if rep == 0 and g == HEADG + 2:
                        nc.sync.dma_start(v_sb[:, VSPLIT:],
                                          v_dram[:, VSPLIT:])

                for g, (g0, w, mm, red) in enumerate(groups):
                    pt = psum_pool.tile([128, GROUP], mybir.dt.float32,
                                        tag="ps")
                    for lo, hi, s in mm:
                        rhs = (h_sb[:, VHEAD + g0 + lo:VHEAD + g0 + hi]
                               if g < HEADG else u_tiles[g][:, lo:hi])
                        nc.tensor.matmul(pt[:, lo:hi], lhs(s), rhs,
                                         start=True, stop=True)
                    acc_mode = g >= len(groups) - ACC_TAIL
                    if acc_mode:
                        for lo, hi, s, k in red:
                            nc.scalar.activation(
                                out=pt[:, lo:hi], in_=pt[:, lo:hi],
                                func=mybir.ActivationFunctionType.Exp,
                                accum_out=parts[:, s, k:k + 1])
                    else:
                        sc = scratch_pool.tile([128, GROUP],
                                               mybir.dt.float16, tag="sc")
                        nc.scalar.activation(
                            out=sc[:, :w], in_=pt[:, :w],
                            func=mybir.ActivationFunctionType.Exp)
                        for lo, hi, s, k in red:
                            L = halve(sc, lo, hi - lo)
                            nc.vector.reduce_sum(parts[:, s, k:k + 1],
                                                 sc[:, lo:lo + L],
                                                 axis=mybir.AxisListType.X)
            nc.vector.reduce_sum(y_sb[:], parts[:],
                                 axis=mybir.AxisListType.X)
            nc.sync.dma_start(y_dram[:], y_sb[:])
    nc.finalize()
    return nc


# ---------------------------------------------------------------------------
# Host orchestration
# ---------------------------------------------------------------------------

def _prep_inputs(rho, gamma, coords, weights):
    rho = np.asarray(rho, np.float64)
    gamma = np.asarray(gamma, np.float64)
    coords = np.asarray(coords, np.float64)
    weights = np.asarray(weights, np.float64)
    Vb, Ub, dead, a, b, lnf, lnn = build_split_vu(rho, gamma, coords, weights)
    KK = Vb.shape[0]
    blocks, percore, sched = _plan(coords, a, b, lnf, lnn)
    tot, groups, maxp = _pieces(sched)

    Ub16 = Ub.astype(ml_dtypes.bfloat16)
    Vb16 = Vb.astype(ml_dtypes.bfloat16)
    dead16 = dead.astype(ml_dtypes.bfloat16)

    assert sched[0] >= FIRSTG
    in_maps = []
    for m in range(N_CORES):
        u = np.empty((KK, tot), ml_dtypes.bfloat16)
        v = np.empty((KK, SLOTS * BLK), ml_dtypes.bfloat16)
        col = 0
        for s, (pidx, pch, idx) in enumerate(percore[m]):
            w = sched[s]
            u[:, col:col + len(idx)] = Ub16[:, idx]
            u[:, col + len(idx):col + w] = dead16[:, None]
            col += w
            v[:, s * BLK:(s + 1) * BLK] = Vb16[:, pidx, pch]
        h = np.concatenate([v[:, :VHEAD], u[:, :HEADU]], axis=1)
        in_maps.append({"h": np.ascontiguousarray(h), "u": u, "v": v})
    meta = (blocks, percore)
    return KK, tot, groups, maxp, in_maps, meta


def _assemble(results, meta):
    blocks, percore = meta
    out = np.empty((N, 3), np.float32)
    for m, res in enumerate(results):
        y_dev = np.asarray(res["y"])                   # [128, SLOTS]
        for s, (pidx, pch, idx) in enumerate(percore[m]):
            out[pidx, pch] = y_dev[:, s]
    return out


def kernel_run(rho, gamma, coords, weights, **spmd_kwargs):
    from concourse.bass_utils import run_bass_kernel_spmd

    KK, tot, groups, maxp, in_maps, meta = _prep_inputs(
        rho, gamma, coords, weights)
    key = (KK, tot, maxp, tuple(g[0] for g in groups))
    if key not in _NC_CACHE:
        _NC_CACHE[key] = _build_nc(KK, tot, groups, maxp)
    res = run_bass_kernel_spmd(_NC_CACHE[key], in_maps,
                               core_ids=list(range(N_CORES)), **spmd_kwargs)
    return _assemble(res.results, meta), res


def kernel(rho, gamma, coords, weights):
    y, _ = kernel_run(rho, gamma, coords, weights)
    return y


# revision 51
# speedup vs baseline: 1.0714x; 1.0045x over previous
"""Trainium2 Bass kernel for nn_CiderFeatures (all-pairs Gaussian reduction).

y[i, c] = norms[c] * sum_j exp(-(a_j + b[i,c]) * ||x_i - x_j||^2) * f_j

with per-point scalars a, b, f derived from (rho, gamma, weights).

Strategy (8 NeuronCores, spatially block-sparse row-parallel):
  - The exp argument is a bilinear form: arg[ic, j] = V[:, ic] . U[:, j]
    (10 logical dims expanding -(a_j + b_ic)(r_i + r_j - 2 x_i.x_j)
    + ln f_j + ln norms_c), split into bf16 hi/mid/lo levels (K~54) so
    bf16 matmuls recover ~fp32 precision at full PE speed.
  - Spatial sparsity: the Gaussian factor underflows for all but ~5% of
    pairs.  Points are sorted into 128 tight blocks of 128 via recursive
    coordinate bisection; for each (block, channel) a conservative bound
    keeps only j with exp(-(a_j + bmin_Ic) d2min(I, x_j) + lnf_j + lnn_c)
    >= TAU.  Kept columns are host-gathered into a packed U per core.
  - The 384 (block, channel) units are LPT-balanced across 8 cores x 48
    slots with a shared per-slot width schedule (padded with dead columns
    whose exp underflows to 0), so all cores run one identical program.
  - Device: stream packed U in 2048-col groups (a fused head DMA carries
    the first slots' V plus group 0 to shorten startup); TensorE matmuls
    chopped at PSUM-bank/slot boundaries; ScalarE exp -> fp16 SBUF
    scratch; VectorE in-place halving adds (2x 16-bit mode) + reduce per
    slot-piece; one final 3D reduce -> y.  The trailing groups use the
    ACT accumulator for the j-sum so VectorE does not extend the tail.
"""

import numpy as np
import ml_dtypes
from math import pi, ceil, log

N = 16384
N_CORES = 8
NB = 128                 # spatial blocks
BLK = 128                # points per block (partition dim)
SLOTS = NB * 3 // N_CORES  # 48 (block, channel) units per core
GROUP = 2048             # exp/psum group width (4 PSUM banks)
FIRSTG = 512             # width of the first (startup) group
SECONDG = 1536           # width of the second group
HEADG = 1                # leading groups whose u columns ride the head DMA
HEADU = FIRSTG
VHEAD = 4 * BLK          # v columns carried by the head DMA (slots 0-3)
BANK = 512               # PSUM bank width in fp32
RTOL2 = 2.8e-4           # keep j for (block, channel) iff some row i has
                         # exp(arg_ij) >= RTOL2 * ytilde_i (exact host test;
                         # ytilde is a per-row lower bound on y)
DEAD_ARG = -300.0        # exponent of dead padding columns (exp -> 0)
LNF_FLOOR = -100.0

SPLIT_LEVELS = 3
MAX_LEVEL_SUM = 2

# keep halving while the pre-halving even length is >= HMIN (each level
# costs ~60ns busy overhead and saves L/2 * 0.52ns of reduce time)
HMIN = 224
# trailing groups whose j-sum uses the ACT accumulator instead of DVE, so
# DVE does not finish last and extend the tail
ACC_TAIL = 2
# dummy matmuls that pre-ramp the PE p-state during the startup DMA window
NDUMMY = 4
# slot widths are rounded up to this multiple (keeps halving-add lengths even)
WROUND = 16


# ---------------------------------------------------------------------------
# Host math (float64): derived scalars and the bilinear decomposition
# ---------------------------------------------------------------------------

def _derived(rho, gamma, weights, coords):
    A, D = 2.0, 2.0
    B2, C2 = A, (6.0 * pi ** 2) ** (2.0 / 3.0) * (6.0 * A / (160.0 * pi))
    B3, C3 = 2.0 * B2, 2.0 * C2
    B0, C0 = D / A * B2, D / A * C2
    B1, C1 = B2 / 2.0, C2 / 2.0
    Bs = np.array([B0, B1, B2, B3])
    Cs = np.array([C0, C1, C2, C3])
    norms = ((Bs[0] + Bs[1:]) / 2.0) ** 1.5  # (3,)

    rho_ = rho + 1e-8
    t_w = gamma / (8.0 * rho_)
    t_tf = 0.3 * (3.0 * pi ** 2) ** (2.0 / 3.0) * rho_ ** (5.0 / 3.0)
    x = t_w / t_tf
    scale = pi * (rho_ / 2.0) ** (2.0 / 3.0)
    ab = scale[:, None] * (Bs[None, :] + Cs[None, :] * x[:, None])  # (N,4)
    a = ab[:, 0]
    b = ab[:, 1:]                                                   # (N,3)
    f = weights * rho
    lnf = np.log(np.maximum(f, 1e-300))
    lnf = np.maximum(lnf, LNF_FLOOR)
    r = np.sum(coords * coords, axis=1)
    return a, b, f, lnf, r, norms


def _build_vu10(rho, gamma, coords, weights):
    """arg[ic, j] = sum_k V10[k, i, c] * U10[k, j]; a and r mean-centered."""
    a, b, f, lnf, r, norms = _derived(rho, gamma, weights, coords)
    lnn = np.log(norms)
    rbar = float(r.mean())
    rc = r - rbar
    abar = float(a.mean())
    ac = a - abar
    xyz = coords

    V10 = np.empty((10, N, 3))
    U10 = np.empty((10, N))
    V10[0] = np.broadcast_to(rc[:, None], (N, 3))
    U10[0] = -ac
    V10[1] = 1.0
    U10[1] = -a * r + lnf - ac * rbar
    V10[2:5] = np.broadcast_to((2.0 * xyz).T[:, :, None], (3, N, 3))
    U10[2:5] = (ac[:, None] * xyz).T
    V10[5] = b
    U10[5] = -rc
    V10[6] = (-(b * (r[:, None] + rbar))
              - abar * (rc[:, None] + rbar)
              + lnn[None, :])
    U10[6] = 1.0
    V10[7:10] = np.moveaxis(
        2.0 * (b + abar)[:, :, None] * xyz[:, None, :], 2, 0)
    U10[7:10] = xyz.T
    return V10, U10, a, b, lnf, lnn


def _bf16_levels(M, nlev):
    rem = M.copy()
    outs = []
    for _ in range(nlev):
        h = np.asarray(rem, ml_dtypes.bfloat16).astype(np.float64)
        outs.append(h)
        rem = rem - h
    return outs


def build_split_vu(rho, gamma, coords, weights,
                   nlev=SPLIT_LEVELS, max_sum=MAX_LEVEL_SUM):
    """bf16-split V/U plus the scalars needed for the sparsity bound.

    Returns (Vb [K,N,3], Ub [K,N], dead [K], a, b, lnf, lnn) where `dead`
    is a padding U column whose arg is DEAD_ARG for every (i, c).
    """
    V10, U10, a, b, lnf, lnn = _build_vu10(rho, gamma, coords, weights)
    Vlev = [_bf16_levels(V10[d], nlev) for d in range(10)]
    Ulev = [_bf16_levels(U10[d], nlev) for d in range(10)]

    vrows, urows, drows = [], [], []
    for s in range(max_sum + 1):
        for d in range(10):
            for lv in range(min(s, nlev - 1) + 1):
                lu = s - lv
                if lu >= nlev:
                    continue
                v = Vlev[d][lv]
                u = Ulev[d][lu]
                if not v.any() or not u.any():
                    continue
                vrows.append(v)
                urows.append(u)
                # dead col: ride the (d=1, lv=0) pure-j row (V == 1.0)
                drows.append(DEAD_ARG if (d == 1 and lv == 0 and s == 0)
                             else 0.0)
    Vb = np.stack(vrows).astype(np.float32)   # [K, N, 3]
    Ub = np.stack(urows).astype(np.float32)   # [K, N]
    dead = np.array(drows, np.float32)        # [K]
    return Vb, Ub, dead, a, b, lnf, lnn


# ---------------------------------------------------------------------------
# Spatial blocking, sparsity bound, core assignment, schedule
# ---------------------------------------------------------------------------

def _rcb(coords, idx, nblocks):
    if nblocks == 1:
        return [idx]
    pts = coords[idx]
    dim = int(np.argmax(pts.max(0) - pts.min(0)))
    order = np.argsort(pts[:, dim], kind="stable")
    h = len(idx) // 2
    return (_rcb(coords, idx[order[:h]], nblocks // 2)
            + _rcb(coords, idx[order[h:]], nblocks // 2))


def _plan(coords, a, b, lnf, lnn, rtol=RTOL2):
    """Blocks, kept-j lists per (block, channel), core assignment, schedule.

    Two-stage keep test per (block I, channel c):
      1. conservative per-sub-box bbox bound (superset, vectorized)
      2. exact max_i exp(arg_ij) >= rtol * ytilde_i on the candidates

    Returns (blocks, units_per_core, sched) where units_per_core[m] is a
    list of SLOTS tuples (I, c, idx_array) in slot order and sched[s] is
    the shared padded column count of slot s.
    """
    blocks = _rcb(coords, np.arange(N), NB)
    SUB = 16                                  # sub-boxes per block
    SS = BLK // SUB

    # ytilde[i, c]: within-block partial sum — a true lower bound on y
    ytilde = np.zeros((N, 3))
    for blk in blocks:
        d2 = ((coords[blk][:, None, :] - coords[blk][None, :, :]) ** 2).sum(-1)
        for c in range(3):
            w = np.exp(-(a[blk][None, :] + b[blk, c][:, None]) * d2
                       + lnf[blk][None, :] + lnn[c])
            ytilde[blk, c] = w.sum(1)

    r2 = (coords * coords).sum(1)

    def exact_keep(rows, c, cand):
        d2 = (r2[rows][:, None] + r2[cand][None, :]
              - 2.0 * (coords[rows] @ coords[cand].T))
        arg = (-(a[cand][None, :] + b[rows, c][:, None])
               * np.maximum(d2, 0.0) + lnf[cand][None, :] + lnn[c])
        return (np.exp(arg) / ytilde[rows, c][:, None] >= rtol).any(0)

    def stage1(rows, cb, cn, T):
        """Candidate superset: bound with channel-cb exponents, channel-cn
        norm (valid for any channel c with b_c >= b_cb, lnn_c <= lnn_cn)."""
        keep = np.zeros(N, bool)
        nsub = max(1, len(rows) // SS)
        for s in range(nsub):
            sub = rows[s * SS:(s + 1) * SS]
            d = np.maximum(0.0, np.maximum(coords[sub].min(0)[None, :]
                                           - coords,
                                           coords - coords[sub].max(0)[None, :]))
            d2 = (d * d).sum(1)
            keep |= (a + b[sub, cb].min()) * d2 - lnf - lnn[cn] < T
        return keep

    # Units pack 128 partitions as (point, channel) pairs:
    #  - two units per block of (64-point half) x channels {0, 1} — the
    #    union keep-set spans a smaller radius than full-block units, and
    #    b1 = 2*b0 exactly so the ch0 bound covers ch1
    #  - one unit per block of (128 points) x channel 2
    units = []
    for I, blk in enumerate(blocks):
        for half in (blk[:64], blk[64:]):
            T = -log(rtol * ytilde[half, :2].min())
            cand = np.flatnonzero(stage1(half, 0, 1, T))
            keep = exact_keep(half, 0, cand) | exact_keep(half, 1, cand)
            pidx = np.concatenate([half, half])
            pch = np.array([0] * 64 + [1] * 64)
            units.append((pidx, pch, cand[keep]))
        T = -log(rtol * ytilde[blk, 2].min())
        cand = np.flatnonzero(stage1(blk, 2, 2, T))
        units.append((blk, np.full(BLK, 2), cand[exact_keep(blk, 2, cand)]))

    # LPT assignment, capped at SLOTS units per core
    units.sort(key=lambda u: -len(u[2]))
    loads = [0] * N_CORES
    counts = [0] * N_CORES
    percore = [[] for _ in range(N_CORES)]
    for u in units:
        m = min((m for m in range(N_CORES) if counts[m] < SLOTS),
                key=lambda m: loads[m])
        percore[m].append(u)
        loads[m] += len(u[2])
        counts[m] += 1
    # Slot order: valley shape (widest slots at both ends of the stream).
    # Group 0 then only needs the head of v, and the trailing groups have
    # few pieces, keeping the ACT-accumulated tail cheap.
    perm = list(range(0, SLOTS, 2)) + list(range(SLOTS - 1 - (SLOTS % 2), 0, -2))
    for m in range(N_CORES):
        percore[m].sort(key=lambda u: -len(u[2]))
        percore[m] = [percore[m][p] for p in perm]

    sched = []
    for s in range(SLOTS):
        w = max(len(percore[m][s][2]) for m in range(N_CORES))
        sched.append(-WROUND * (-w // WROUND))
    return blocks, percore, sched


def _pieces(sched):
    """Static program structure from the shared schedule.

    Returns (tot, groups) where groups[g] = (g0, w, mm, red):
      mm  = list of (lo, hi, slot) matmul pieces (group-local, bank-aligned)
      red = list of (lo, hi, pcol) reduce pieces (group-local, slot runs)
    and pcol indexes parts as slot * MAXP + k.
    """
    tot = sum(sched)
    edges = np.cumsum([0] + list(sched))
    # first group is small so the initial DMA + matmul + exp chain starts
    # as early as possible
    bounds = [0, FIRSTG, FIRSTG + SECONDG]
    while bounds[-1] < tot:
        bounds.append(min(bounds[-1] + GROUP, tot))
    bounds = sorted(set(min(b, tot) for b in bounds))
    npieces = [0] * SLOTS
    groups = []
    for g in range(len(bounds) - 1):
        g0, g1 = bounds[g], bounds[g + 1]
        mm, red = [], []
        for s in range(SLOTS):
            lo, hi = max(edges[s], g0), min(edges[s + 1], g1)
            if lo >= hi:
                continue
            red.append((lo - g0, hi - g0, s, npieces[s]))
            npieces[s] += 1
            p = lo
            while p < hi:
                q = min(hi, (p // BANK + 1) * BANK)
                mm.append((p - g0, q - g0, s))
                p = q
        groups.append((g0, g1 - g0, mm, red))
    maxp = max(npieces)
    return tot, groups, maxp


# ---------------------------------------------------------------------------
# Device kernel
# ---------------------------------------------------------------------------

_NC_CACHE = {}


def _build_nc(KK, tot, groups, maxp, repeat=1):
    import concourse.bass as bass  # noqa: F401
    import concourse.tile as tile
    from concourse import bacc, mybir

    nc = bacc.Bacc("TRN2", target_bir_lowering=False)
    # h carries the v columns of slots 0-3 plus the u columns of the first
    # HEADG groups, so a single small DMA unblocks the whole startup chain
    h_dram = nc.dram_tensor("h", [KK, VHEAD + HEADU], mybir.dt.bfloat16,
                            kind="ExternalInput")
    u_dram = nc.dram_tensor("u", [KK, tot], mybir.dt.bfloat16,
                            kind="ExternalInput")
    v_dram = nc.dram_tensor("v", [KK, SLOTS * BLK], mybir.dt.bfloat16,
                            kind="ExternalInput")
    y_dram = nc.dram_tensor("y", [BLK, SLOTS], mybir.dt.float32,
                            kind="ExternalOutput")

    def halve(sc, lo, L):
        """In-place halving adds on scratch; returns surviving length."""
        while L % 2 == 0 and L >= HMIN and L >= 2:
            h = L // 2
            nc.vector.add_instruction(
                mybir.InstTensorTensor(
                    name=nc.get_next_instruction_name(),
                    op=mybir.AluOpType.add,
                    ins=[nc.vector.lower_ap(sc[:, lo:lo + h]),
                         nc.vector.lower_ap(sc[:, lo + h:lo + 2 * h])],
                    outs=[nc.vector.lower_ap(sc[:, lo:lo + h])],
                ))
            L = h
        return L

    with tile.TileContext(nc) as tc:
        with (
            tc.tile_pool(name="singles", bufs=1) as singles,
            tc.tile_pool(name="upool", bufs=6) as upool,
            tc.tile_pool(name="psum", bufs=2, space="PSUM") as psum_pool,
            tc.tile_pool(name="scratch", bufs=5) as scratch_pool,
        ):
            # warm the ACT exp table during the input-DMA window
            warm = singles.tile([128, 1], mybir.dt.float32)
            nc.vector.memset(warm[:], 0.0)
            nc.scalar.activation(out=warm[:], in_=warm[:],
                                 func=mybir.ActivationFunctionType.Exp)

            # prime the PE p-state ramp during the same window: ~3us of
            # back-to-back dummy matmuls so the first real groups run at
            # full clock
            prime = singles.tile([KK, 640], mybir.dt.bfloat16)
            nc.vector.memset(prime[:], 0.0)
            pdummy = psum_pool.tile([128, GROUP], mybir.dt.float32, tag="ps")
            for _ in range(NDUMMY):
                nc.tensor.matmul(pdummy[:, :512], prime[:, :128],
                                 prime[:, 128:640], start=True, stop=True)

            # head DMA (v slots 0-3 + u groups 0..HEADG-1) goes out first;
            # the v tail streams in two chunks interleaved with the u groups
            h_sb = singles.tile([KK, VHEAD + HEADU], mybir.dt.bfloat16)
            v_sb = singles.tile([KK, SLOTS * BLK], mybir.dt.bfloat16)
            VSPLIT = 16 * BLK
            parts = singles.tile([128, SLOTS, maxp], mybir.dt.float32)
            nc.vector.memset(parts[:], 0.0)
            y_sb = singles.tile([128, SLOTS], mybir.dt.float32)

            def lhs(s):
                if s < 4:
                    return h_sb[:, s * BLK:(s + 1) * BLK]
                return v_sb[:, s * BLK:(s + 1) * BLK]

            for rep in range(repeat):
                u_tiles = [None] * HEADG
                if rep == 0:
                    nc.sync.dma_start(h_sb[:], h_dram[:])
                for g, (g0, w, mm, red) in enumerate(groups[HEADG:], HEADG):
                    ut = upool.tile([KK, GROUP], mybir.dt.bfloat16, tag="u")
                    nc.sync.dma_start(ut[:, :w], u_dram[:, g0:g0 + w])
                    u_tiles.append(ut)
                    if rep == 0 and g == HEADG + 1:
                        nc.sync.dma_start(v_sb[:, VHEAD:VSPLIT],
                                          v_dram[:, VHEAD:VSPLIT])
                    if rep == 0 and g == HEADG + 2:
                        nc.sync.dma_start(v_sb[:, VSPLIT:],
                                          v_dram[:, VSPLIT:])

                for g, (g0, w, mm, red) in enumerate(groups):
                    pt = psum_pool.tile([128, GROUP], mybir.dt.float32,
                                        tag="ps")
                    for lo, hi, s in mm:
                        rhs = (h_sb[:, VHEAD + g0 + lo:VHEAD + g0 + hi]
                               if g < HEADG else u_tiles[g][:, lo:hi])
                        nc.tensor.matmul(pt[:, lo:hi], lhs(s), rhs,
                                         start=True, stop=True)
                    acc_mode = g >= len(groups) - ACC_TAIL
                    if acc_mode:
                        for lo, hi, s, k in red:
                            nc.scalar.activation(
                                out=pt[:, lo:hi], in_=pt[:, lo:hi],
                                func=mybir.ActivationFunctionType.Exp,
                                accum_out=parts[:, s, k:k + 1])
                    else:
                        sc = scratch_pool.tile([128, GROUP],
                                               mybir.dt.float16, tag="sc")
                        nc.scalar.activation(
                            out=sc[:, :w], in_=pt[:, :w],
                            func=mybir.ActivationFunctionType.Exp)
                        for lo, hi, s, k in red:
                            L = halve(sc, lo, hi - lo)
                            nc.vector.reduce_sum(parts[:, s, k:k + 1],
                                                 sc[:, lo:lo + L],
                                                 axis=mybir.AxisListType.X)
            nc.vector.reduce_sum(y_sb[:], parts[:],
                                 axis=mybir.AxisListType.X)
            nc.sync.dma_start(y_dram[:], y_sb[:])
    nc.finalize()
    return nc


# ---------------------------------------------------------------------------
# Host orchestration
# ---------------------------------------------------------------------------

def _prep_inputs(rho, gamma, coords, weights):
    rho = np.asarray(rho, np.float64)
    gamma = np.asarray(gamma, np.float64)
    coords = np.asarray(coords, np.float64)
    weights = np.asarray(weights, np.float64)
    Vb, Ub, dead, a, b, lnf, lnn = build_split_vu(rho, gamma, coords, weights)
    KK = Vb.shape[0]
    blocks, percore, sched = _plan(coords, a, b, lnf, lnn)
    tot, groups, maxp = _pieces(sched)

    Ub16 = Ub.astype(ml_dtypes.bfloat16)
    Vb16 = Vb.astype(ml_dtypes.bfloat16)
    dead16 = dead.astype(ml_dtypes.bfloat16)

    assert sched[0] >= FIRSTG
    in_maps = []
    for m in range(N_CORES):
        u = np.empty((KK, tot), ml_dtypes.bfloat16)
        v = np.empty((KK, SLOTS * BLK), ml_dtypes.bfloat16)
        col = 0
        for s, (pidx, pch, idx) in enumerate(percore[m]):
            w = sched[s]
            u[:, col:col + len(idx)] = Ub16[:, idx]
            u[:, col + len(idx):col + w] = dead16[:, None]
            col += w
            v[:, s * BLK:(s + 1) * BLK] = Vb16[:, pidx, pch]
        h = np.concatenate([v[:, :VHEAD], u[:, :HEADU]], axis=1)
        in_maps.append({"h": np.ascontiguousarray(h), "u": u, "v": v})
    meta = (blocks, percore)
    return KK, tot, groups, maxp, in_maps, meta


def _assemble(results, meta):
    blocks, percore = meta
    out = np.empty((N, 3), np.float32)
    for m, res in enumerate(results):
        y_dev = np.asarray(res["y"])                   # [128, SLOTS]
        for s, (pidx, pch, idx) in enumerate(percore[m]):
            out[pidx, pch] = y_dev[:, s]
    return out


def kernel_run(rho, gamma, coords, weights, **spmd_kwargs):
    from concourse.bass_utils import run_bass_kernel_spmd

    KK, tot, groups, maxp, in_maps, meta = _prep_inputs(
        rho, gamma, coords, weights)
    key = (KK, tot, maxp, tuple(g[0] for g in groups))
    if key not in _NC_CACHE:
        _NC_CACHE[key] = _build_nc(KK, tot, groups, maxp)
    res = run_bass_kernel_spmd(_NC_CACHE[key], in_maps,
                               core_ids=list(range(N_CORES)), **spmd_kwargs)
    return _assemble(res.results, meta), res


def kernel(rho, gamma, coords, weights):
    y, _ = kernel_run(rho, gamma, coords, weights)
    return y
